# revision 47
# baseline (speedup 1.0000x reference)
"""Fused causal MHA kernel for TRN2, one core = (batch b, head-group g of 8 heads).

v2: fp8-e4m3 DoubleRow projections + big DMAs + chunk-major schedule.

Layouts (per core):
  xt8   [128, (cc=4, kb=4, i=2, n=512)] fp8   X[b]^T in DoubleRow-paired
        k-blocks: element (p, cc, kb, i, n) = X[cc*512+n, kb*256+i*128+p].
        DMA'd per-cc (4 contiguous 512KB transfers).
  wq8/wk8/wv8 [128, (kb=4, i=2, m=512)] fp8   32*W column shard, paired.
  wo    [128, (dv=4, do=1024)] f16            row shard, dv on partitions.
  mask01 [128, 128] f16  triangle: mask01[j, i] = 1 if i >= j else 0.
  outt  [1024, N] f16    partial (X attn Wo_g)^T ; host sums the two
        head-group partials per batch (f32) and transposes.

Projections run in fp8 DoubleRow mode (contraction 256/instr, 2x rate):
  psQ = lhsT.T@rhs summed over the paired dim; W pre-scaled by 32 on host
  to clear e4m3 subnormals, un-scaled in the psum->sbuf copy.
Attention (S, PV) and the out-projection stay fp16.

On-chip attention per head-pair hp (2 heads):
  qt/kt [128, N] f16; partitions = (h0 d0-63, h1 d0-63); qt carries /8/32.
  v per seq m-block: [128, 8*65]; seq on partitions, (64 dims + ones) per head.
  S^T per (hp, c, jb): psum [128, 1024] = h0|h1; j on partitions, i on free.
  exp runs directly on psS (no mask add); the causal triangle is zeroed
  after exp with a fp16 multiply by mask01 (diagonal blocks only).
  PV col-packed: psO[0:65] per head (65th row = denominator via ones col).

Schedule is chunk-major: for c in 0..3: for hp in 0..3, so that Q/K
projection of chunk c+1 and the out-projection of chunk c-1 are always
available as pump work to fill the PE while the ACT engine (exp) catches up.
"""

import numpy as np
import concourse.bass as bass
import concourse.tile as tile
from concourse import bacc, mybir

F32 = mybir.dt.float32
F16 = mybir.dt.float16
F8 = mybir.dt.float8e4
AF = mybir.ActivationFunctionType
DR = mybir.MatmulPerfMode.DoubleRow

P = 128
D = 1024
DH = 512  # head-group width: 8 heads * 64
DK = 64
KB2 = 4  # 256-wide paired k-blocks
NHP = 4  # head-pairs per core
WSCALE = 32.0  # host pre-scale on W before fp8 cast


def build(N=2048, interleave=True, debug_inline_qk=False, debug_inline_v=False,
          rescue=True, rescue_level=3):
    MB = N // P  # seq 128-blocks
    MC = N // 512  # seq 512-chunks
    nc = bacc.Bacc("TRN2", target_bir_lowering=False, debug=False)

    xt8_d = nc.dram_tensor("xt8", [MC * P, 4096], F8, kind="ExternalInput")
    wq8_d = nc.dram_tensor("wq8", [P, 4096], F8, kind="ExternalInput")
    wk8_d = nc.dram_tensor("wk8", [P, 4096], F8, kind="ExternalInput")
    wv8_d = nc.dram_tensor("wv8", [P, 4096], F8, kind="ExternalInput")
    wo_d = nc.dram_tensor("wo", [P, 4096], F16, kind="ExternalInput")
    mask_d = nc.dram_tensor("mask01", [P, P], F16, kind="ExternalInput")
    # fp16 rescue inputs: W fp16 (full) + X^T fp16 for seq 0-127; the first
    # 128 output rows are recomputed in fp16 (peaked softmax amplifies the
    # fp8 projection noise there)
    wq16_d = nc.dram_tensor("wq16", [P, 4096], F16, kind="ExternalInput")
    wk16_d = nc.dram_tensor("wk16", [P, 4096], F16, kind="ExternalInput")
    wv16_d = nc.dram_tensor("wv16", [P, 4096], F16, kind="ExternalInput")
    xt16_d = nc.dram_tensor("xt16", [P, 1024], F16, kind="ExternalInput")
    out_d = nc.dram_tensor("outt", [D, N], F16, kind="ExternalOutput")

    with tile.TileContext(nc) as tc:
        with (
            tc.tile_pool(name="sb", bufs=1) as sb,
            tc.tile_pool(name="ps", bufs=1, space="PSUM") as ps,
        ):
            # ---- persistent tiles ----
            ones = sb.tile([P, DK], F16, tag="ones", bufs=1)
            mask01 = sb.tile([P, P], F16, tag="mask01", bufs=1)
            xt8 = sb.tile([P, MC * 4096], F8, tag="xt8", bufs=1)
            wv8 = sb.tile([P, 4096], F8, tag="wv8", bufs=1)
            wq8 = sb.tile([P, 4096], F8, tag="wq8", bufs=1)
            wk8 = sb.tile([P, 4096], F8, tag="wk8", bufs=1)
            wo_t = sb.tile([P, 4096], F16, tag="wo", bufs=1)
            v = [sb.tile([P, 8 * 65], F16, tag="v", bufs=MB, name=f"v{m}") for m in range(MB)]
            qt = [sb.tile([P, N], F16, tag="qt", bufs=NHP, name=f"qt{h}") for h in range(NHP)]
            kt = [sb.tile([P, N], F16, tag="kt", bufs=NHP, name=f"kt{h}") for h in range(NHP)]
            ot = [sb.tile([P, N], F16, tag="ot", bufs=NHP, name=f"ot{t}") for t in range(NHP)]

            # 5-d views: (p, cc, kb, i, n) / (p, kb, i, m)
            x5 = xt8[:].rearrange("p (cc kb i n) -> p cc kb i n", cc=MC, kb=KB2, i=2)
            wq5 = wq8[:].rearrange("p (kb i m) -> p kb i m", kb=KB2, i=2)
            wk5 = wk8[:].rearrange("p (kb i m) -> p kb i m", kb=KB2, i=2)
            wv5 = wv8[:].rearrange("p (kb i m) -> p kb i m", kb=KB2, i=2)
            wo3 = wo_t[:].rearrange("p (dv do) -> p dv do", dv=NHP)

            # ---- input DMAs: few, large, cc-ordered ----
            nc.gpsimd.memset(ones[:], 1.0)
            nc.sync.dma_start(mask01[:], mask_d.ap())
            nc.sync.dma_start(wv8[:], wv8_d.ap())
            for cc in range(MC):
                nc.sync.dma_start(
                    xt8[:, cc * 4096:(cc + 1) * 4096],
                    xt8_d.ap()[cc * P:(cc + 1) * P, :],
                )
            nc.sync.dma_start(wq8[:], wq8_d.ap())
            nc.sync.dma_start(wk8[:], wk8_d.ap())
            nc.sync.dma_start(wo_t[:], wo_d.ap())
            w16q = sb.tile([P, 4096], F16, tag="w16q", bufs=1)
            w16k = sb.tile([P, 4096], F16, tag="w16k", bufs=1)
            w16v = sb.tile([P, 4096], F16, tag="w16v", bufs=1)
            xt16 = sb.tile([P, 1024], F16, tag="xt16", bufs=1)
            nc.sync.dma_start(xt16[:], xt16_d.ap())
            nc.sync.dma_start(w16q[:], wq16_d.ap())
            nc.sync.dma_start(w16k[:], wk16_d.ap())
            nc.sync.dma_start(w16v[:], wv16_d.ap())
            wq316 = w16q[:].rearrange("p (kb m) -> p kb m", kb=8)
            wk316 = w16k[:].rearrange("p (kb m) -> p kb m", kb=8)
            wv316 = w16v[:].rearrange("p (kb m) -> p kb m", kb=8)
            x316 = xt16[:].rearrange("p (kb n) -> p kb n", kb=8)

            # warm the ACT exp table during the DMA lead-in
            warm = sb.tile([P, DK], F16, tag="warm", bufs=1, name="warm")
            nc.scalar.activation(warm[:], ones[:], AF.Exp)
            # warm the PE p-state clock with throwaway matmuls on `ones`
            psW = ps.tile([P, 512], F32, tag="proj", bufs=2, name="psW")
            for _ in range(140):
                nc.tensor.matmul(
                    psW[0:DK, 0:DK], ones[:], ones[:], start=True, stop=True
                )

            # ---- deferred work pumped between attention units ----
            # `deferred` (must-run): drained at each phase boundary so every
            # write is emitted before its next-phase reader. `opt_q`: carries
            # across phases (out-projections); popped only when `deferred`
            # is empty, which also guarantees rescue-before-outproj(0).
            deferred = []
            opt_q = []
            dve_deferred = []
            credit = [0.0]
            hold = [0]

            def pump(rate):
                for _ in range(2):
                    if dve_deferred:
                        dve_deferred.pop(0)()
                credit[0] += rate
                while credit[0] >= 1.0:
                    if deferred:
                        deferred.pop(0)()
                    elif len(opt_q) > hold[0]:
                        opt_q.pop(0)()
                    else:
                        credit[0] = 0.0
                        break
                    credit[0] -= 1.0

            def v_proj(m):
                cc, ms = m // 4, m % 4
                psV = ps.tile([P, 512], F32, tag="proj", bufs=2, name="psV")
                for kb in range(KB2):
                    nc.tensor.matmul(
                        psV[:],
                        x5[:, cc, kb, :, ms * P:(ms + 1) * P],
                        wv5[:, kb, :, :],
                        start=(kb == 0),
                        stop=(kb == KB2 - 1),
                        perf_mode=DR,
                    )
                v3 = v[m][:].rearrange("p (h x) -> p h x", x=65)
                nc.scalar.mul(
                    v3[:, :, 0:64], psV[:].rearrange("p (h x) -> p h x", x=64),
                    1.0 / WSCALE,
                )
                nc.scalar.copy(v3[:, :, 64:65], ones[:, 0:8, None])

            def qk_proj(hp, c, w5, dst, scale):
                # one thunk per projection: a DoubleRow accumulation group
                # must not interleave with foreign (fp16) matmuls
                psQ = ps.tile([P, 512], F32, tag="proj", bufs=2, name="psQ")
                for kb in range(KB2):
                    nc.tensor.matmul(
                        psQ[:],
                        w5[:, kb, :, hp * P:(hp + 1) * P],
                        x5[:, c, kb, :, :],
                        start=(kb == 0),
                        stop=(kb == KB2 - 1),
                        perf_mode=DR,
                    )
                nc.vector.tensor_scalar_mul(
                    dst[:, c * 512:(c + 1) * 512], psQ[:], scale
                )

            def qk_work(hp, c):
                return [
                    lambda: qk_proj(hp, c, wq5, qt[hp], 0.125 / WSCALE),
                    lambda: qk_proj(hp, c, wk5, kt[hp], 1.0 / WSCALE),
                ]

            def attn_chunk(hp, c, pump_rate=0.5, norm_q=None, prepend_norm=False):
                jb_max = min(MB, 4 * c + 4)
                psOa = [
                    ps.tile([65, 512], F32, tag="psO", bufs=2, name="psO0"),
                    ps.tile([65, 512], F32, tag="psO", bufs=2, name="psO1"),
                ]
                pts = {}

                def stage_s(jb):
                    psS = ps.tile([P, 1024], F32, tag="psS", bufs=2, name="psS")
                    r = jb - 4 * c
                    pre = P * r if r > 0 else 0
                    for h2 in range(2):
                        nc.tensor.matmul(
                            psS[:, h2 * 512 + pre:(h2 + 1) * 512],
                            kt[hp][h2 * DK:(h2 + 1) * DK, jb * P:(jb + 1) * P],
                            qt[hp][h2 * DK:(h2 + 1) * DK, c * 512 + pre:(c + 1) * 512],
                            start=True,
                            stop=True,
                            tile_position=(h2 * DK, 0),
                        )
                    pt = sb.tile([P, 1024], F16, tag="pt", bufs=6, name="pt")
                    if pre:
                        # exp only the valid tail per head; zero the prefix
                        psS3 = psS[:].rearrange("p (h x) -> p h x", h=2)
                        pt3 = pt[:].rearrange("p (h x) -> p h x", h=2)
                        nc.scalar.activation(
                            pt3[:, :, pre:512], psS3[:, :, pre:512], AF.Exp
                        )
                        nc.gpsimd.memset(pt3[:, :, 0:pre], 0.0)
                    else:
                        nc.scalar.activation(pt[:], psS[:], AF.Exp)
                    if r >= 0:
                        # zero the strictly-upper part of the 128-wide
                        # diagonal triangle (post-exp 0/1 multiply)
                        for h2 in range(2):
                            nc.vector.tensor_tensor(
                                pt[:, h2 * 512 + pre:h2 * 512 + pre + P],
                                pt[:, h2 * 512 + pre:h2 * 512 + pre + P],
                                mask01[:],
                                mybir.AluOpType.mult,
                            )
                    pts[jb] = pt

                def stage_pv(jb):
                    pt = pts.pop(jb)
                    first, last = (jb == 0), (jb == jb_max - 1)
                    r = jb - 4 * c
                    pre = P * r if (r > 0 and not first) else 0
                    for h2 in range(2):
                        h = 2 * hp + h2
                        nc.tensor.matmul(
                            psOa[h2][0:65, pre:512],
                            v[jb][:, h * 65:(h + 1) * 65],
                            pt[:, h2 * 512 + pre:(h2 + 1) * 512],
                            start=first,
                            stop=last,
                            skip_group_check=True,
                        )
                    pump(pump_rate)

                for jb in range(jb_max):
                    stage_s(jb)
                    if jb >= 2:
                        stage_pv(jb - 2)
                stage_pv(jb_max - 2)
                stage_pv(jb_max - 1)

                cpO = [
                    sb.tile([65, 512], F32, tag="sm512", bufs=14, name=f"cpO{h2}")
                    for h2 in range(2)
                ]
                nc.vector.tensor_copy(cpO[0][0:65, :], psOa[0][0:65, :])
                nc.vector.tensor_copy(cpO[1][0:65, :], psOa[1][0:65, :])
                rbc = [
                    sb.tile([64, 512], F32, tag="sm512", bufs=14, name=f"rbc{h2}")
                    for h2 in range(2)
                ]
                tmp1 = sb.tile([64, 512], F16, tag="sm512h", bufs=8, name="tmp1")

                nr = sb.tile([1, 1024], F32, tag="nr", bufs=4, name="nr")

                def norm_piece(stage):
                    if stage == 0:
                        # move denominator rows (lane 64) to lane 0, then
                        # broadcast (HW broadcast source must be lane 0)
                        nc.sync.dma_start(nr[0:1, 0:512], cpO[0][64:65, :])
                        nc.sync.dma_start(nr[0:1, 512:1024], cpO[1][64:65, :])
                        nc.gpsimd.partition_broadcast(
                            rbc[0][0:64, :], nr[0:1, 0:512]
                        )
                        nc.gpsimd.partition_broadcast(
                            rbc[1][0:64, :], nr[0:1, 512:1024]
                        )
                    elif stage == 1:
                        nc.vector.reciprocal_approx_fast(
                            rbc[0][0:64, :], rbc[0][0:64, :]
                        )
                        nc.vector.reciprocal_approx_fast(
                            rbc[1][0:64, :], rbc[1][0:64, :]
                        )
                    elif stage == 2:
                        nc.vector.tensor_tensor(
                            ot[hp][0:64, c * 512:(c + 1) * 512],
                            cpO[0][0:64, :],
                            rbc[0][0:64, :],
                            mybir.AluOpType.mult,
                        )
                    elif stage == 3:
                        nc.vector.tensor_tensor(
                            tmp1[0:64, :],
                            cpO[1][0:64, :],
                            rbc[1][0:64, :],
                            mybir.AluOpType.mult,
                        )
                        nc.sync.dma_start(
                            ot[hp][64:128, c * 512:(c + 1) * 512], tmp1[0:64, :]
                        )

                if interleave and not prepend_norm:
                    for st in range(4):
                        dve_deferred.append(lambda st=st: norm_piece(st))
                else:
                    for st in range(4):
                        norm_piece(st)

            # ---- fp16 rescue of output rows 0-127 (recompute with fp16
            # Q/K/V; overwrites ot[:, 0:128] before outproj(0) reads it) ----
            rescue_state = {}

            def rescue_thunks():
                v16 = sb.tile([P, 8 * 65], F16, tag="v16", bufs=1, name="v16")
                qt16 = [
                    sb.tile([P, P], F16, tag="qk16", bufs=8, name=f"qt16_{h}")
                    for h in range(NHP)
                ]
                kt16 = [
                    sb.tile([P, P], F16, tag="qk16", bufs=8, name=f"kt16_{h}")
                    for h in range(NHP)
                ]

                def v16_proj():
                    psR = ps.tile([P, 512], F32, tag="proj", bufs=2, name="psR")
                    for kb in range(8):
                        nc.tensor.matmul(
                            psR[:],
                            x316[:, kb, :],
                            wv316[:, kb, :],
                            start=(kb == 0),
                            stop=(kb == 7),
                        )
                    v3 = v16[:].rearrange("p (h x) -> p h x", x=65)
                    nc.vector.tensor_copy(
                        v3[:, :, 0:64], psR[:].rearrange("p (h x) -> p h x", x=64)
                    )
                    nc.vector.tensor_copy(v3[:, :, 64:65], ones[:, 0:8, None])

                def qk16_proj(hp):
                    for w3, dst, scale in (
                        (wq316, qt16[hp], 0.125),
                        (wk316, kt16[hp], 1.0),
                    ):
                        psR = ps.tile([P, 512], F32, tag="proj", bufs=2, name="psR")
                        for kb in range(8):
                            nc.tensor.matmul(
                                psR[:, 0:P],
                                w3[:, kb, hp * P:(hp + 1) * P],
                                x316[:, kb, :],
                                start=(kb == 0),
                                stop=(kb == 7),
                            )
                        nc.vector.tensor_scalar_mul(dst[:], psR[:, 0:P], scale)

                def attn16(hp):
                    # concurrent row-band S matmuls must land in different
                    # PSUM banks: h2 offset by 512 f32 columns
                    psA = ps.tile([P, 1024], F32, tag="psS", bufs=2, name="psA")
                    psA3 = psA[:].rearrange("p (h x) -> p h x", h=2)
                    for h2 in range(2):
                        nc.tensor.matmul(
                            psA[:, h2 * 512:h2 * 512 + P],
                            kt16[hp][h2 * DK:(h2 + 1) * DK, :],
                            qt16[hp][h2 * DK:(h2 + 1) * DK, :],
                            start=True,
                            stop=True,
                            tile_position=(h2 * DK, 0),
                        )
                    pt16 = sb.tile([P, 256], F16, tag="pt16", bufs=2, name="pt16")
                    pt3 = pt16[:].rearrange("p (h x) -> p h x", h=2)
                    nc.scalar.activation(pt3[:, :, :], psA3[:, :, 0:P], AF.Exp)
                    for h2 in range(2):
                        nc.vector.tensor_tensor(
                            pt16[:, h2 * P:(h2 + 1) * P],
                            pt16[:, h2 * P:(h2 + 1) * P],
                            mask01[:],
                            mybir.AluOpType.mult,
                        )
                    psB = ps.tile([P, 1024], F32, tag="psS", bufs=2, name="psB")
                    psB3 = psB[:].rearrange("p (h x) -> p h x", h=2)
                    for h2 in range(2):
                        h = 2 * hp + h2
                        nc.tensor.matmul(
                            psB[0:65, h2 * 512:h2 * 512 + P],
                            v16[:, h * 65:(h + 1) * 65],
                            pt16[:, h2 * P:(h2 + 1) * P],
                            start=True,
                            stop=True,
                        )
                    cp16 = sb.tile([65, 256], F32, tag="cp16", bufs=2, name="cp16")
                    cp3 = cp16[:].rearrange("p (h x) -> p h x", h=2)
                    nc.vector.tensor_copy(cp3[0:65, :, :], psB3[0:65, :, 0:P])
                    nr16 = sb.tile([1, 256], F32, tag="nr16", bufs=2, name="nr16")
                    nr16b = sb.tile([1, 256], F32, tag="nr16", bufs=2, name="nr16b")
                    nc.sync.dma_start(nr16[0:1, 0:P], cp16[64:65, 0:P])
                    nc.sync.dma_start(nr16[0:1, P:256], cp16[64:65, P:256])
                    nc.vector.reciprocal_approx_fast(nr16b[:], nr16[:])
                    rb16 = sb.tile([64, 256], F32, tag="rb16", bufs=2, name="rb16")
                    nc.gpsimd.partition_broadcast(rb16[0:64, 0:P], nr16b[0:1, 0:P])
                    nc.gpsimd.partition_broadcast(rb16[0:64, P:256], nr16b[0:1, P:256])
                    nc.vector.tensor_tensor(
                        ot[hp][0:64, 0:P],
                        cp16[0:64, 0:P],
                        rb16[0:64, 0:P],
                        mybir.AluOpType.mult,
                    )
                    t16 = sb.tile([64, P], F16, tag="t16", bufs=2, name="t16")
                    nc.vector.tensor_tensor(
                        t16[0:64, :],
                        cp16[0:64, P:256],
                        rb16[0:64, P:256],
                        mybir.AluOpType.mult,
                    )
                    nc.sync.dma_start(ot[hp][64:128, 0:P], t16[0:64, :])

                out = [v16_proj]
                if rescue_level >= 2:
                    for hp in range(NHP):
                        out.append(lambda hp=hp: qk16_proj(hp))
                if rescue_level >= 3:
                    for hp in range(NHP):
                        out.append(lambda hp=hp: attn16(hp))
                return out

            def outproj_parts(do, c):
                cell = {}

                def part(v0, v1, fin):
                    if v0 == 0:
                        cell["ps"] = ps.tile(
                            [P, 512], F32, tag="proj", bufs=2, name="psF"
                        )
                    psF = cell["ps"]
                    for dv in range(v0, v1):
                        nc.tensor.matmul(
                            psF[:],
                            wo3[:, dv, do * P:(do + 1) * P],
                            ot[dv][:, c * 512:(c + 1) * 512],
                            start=(dv == 0),
                            stop=(dv == NHP - 1),
                        )
                    if fin:
                        o_sb = sb.tile([P, 512], F16, tag="osb", bufs=6, name="o_sb")
                        nc.scalar.copy(o_sb[:], psF[:])
                        nc.sync.dma_start(
                            out_d.ap()[do * P:(do + 1) * P, c * 512:(c + 1) * 512],
                            o_sb[:],
                        )

                return [lambda: part(0, 2, False), lambda: part(2, NHP, True)]

            # ---- schedule (chunk-major) ----
            # eager V for the first chunks (covers the xt8 DMA stream);
            # the rest is pumped during phase c=0/1.
            n_eager_v = MB if debug_inline_v else 6
            for m in range(n_eager_v):
                v_proj(m)
            deferred.extend([lambda m=m: v_proj(m) for m in range(n_eager_v, 8)])
            # Q/K for chunk 0, all head-pairs, inline
            for hp in range(NHP):
                for wfn in qk_work(hp, 0):
                    wfn()

            for c in range(MC):
                # enqueue next chunk's Q/K (+V, rescue) as must-run work;
                # previous chunk's out-projection as carryable work
                if c + 1 < MC:
                    for h in range(NHP):
                        if debug_inline_qk:
                            for wfn in qk_work(h, c + 1):
                                wfn()
                        else:
                            deferred.extend(qk_work(h, c + 1))
                if c in (1, 2) and not debug_inline_v:
                    # V blocks needed two phases ahead
                    deferred.extend(
                        [lambda m=m: v_proj(m)
                         for m in range(4 * c + 4, 4 * c + 8)]
                    )
                if c == 1 and rescue:
                    # fp16 rescue of rows 0-127: must emit before outproj(0)
                    deferred.extend(rescue_thunks())
                if c >= 1:
                    for do in range(D // P):
                        opt_q.extend(outproj_parts(do, c - 1))
                # emission burst: next chunk's Q/K copies land on the ACT
                # queue early so phase c+1's first S-pairs don't wait
                for _ in range(6):
                    if deferred:
                        deferred.pop(0)()
                phase_units = NHP * (4 * c + 4)
                done = 0
                for hp in range(NHP):
                    last = c == MC - 1 and hp == NHP - 1
                    hold[0] = 16 if c == MC - 1 else 0
                    rate = min(
                        3.0,
                        (len(deferred) + 0.5 * len(opt_q))
                        / max(phase_units - done - 4, 1)
                        + 0.5,
                    )
                    attn_chunk(hp, c, pump_rate=rate,
                               norm_q=None,
                               prepend_norm=last)
                    done += 4 * c + 4
                    if not interleave:
                        while deferred:
                            deferred.pop(0)()
                        while opt_q:
                            opt_q.pop(0)()
                # phase-boundary drain of must-run work only
                if c < MC - 1:
                    while dve_deferred:
                        dve_deferred.pop(0)()
                    while deferred:
                        deferred.pop(0)()

            # ---- drain remaining deferred work, then the last out-proj ----
            while dve_deferred:
                dve_deferred.pop(0)()
            while deferred:
                deferred.pop(0)()
            while opt_q:
                opt_q.pop(0)()
            for do in range(D // P):
                for th in outproj_parts(do, MC - 1):
                    th()

    nc.compile()
    return nc


def make_core_inputs(X, mask, Wq, Wk, Wv, Wo):
    """Full inputs -> list of 8 per-core input maps (batch-major, head-group minor)."""
    import ml_dtypes

    E4M3 = ml_dtypes.float8_e4m3fn
    B = X.shape[0]
    N = X.shape[1]
    MC = N // 512
    mask01 = np.ascontiguousarray(np.triu(np.ones((P, P), np.float16)))

    def pack_x(xt8):  # xt8: [1024, N] fp8 = X[b].T
        # (kb, i, p, cc, n) -> (cc, p, kb, i, n) -> [(cc p), (kb i n)]
        return np.ascontiguousarray(
            xt8.reshape(KB2, 2, P, MC, 512)
            .transpose(3, 2, 0, 1, 4)
            .reshape(MC * P, 4096)
        )

    def pack_w(w8):  # w8: [1024, 512] fp8 = 32*W column shard
        return np.ascontiguousarray(
            w8.reshape(KB2, 2, P, 512).transpose(2, 0, 1, 3).reshape(P, 4096)
        )

    def pack_wo(wos):  # wos: [512, 1024] f16
        return np.ascontiguousarray(
            wos.reshape(NHP, P, D).transpose(1, 0, 2).reshape(P, 4096)
        )

    def pack_w16(w):  # w: [1024, 512] f32 -> [128, (kb=8, m=512)] f16
        return np.ascontiguousarray(
            w.astype(np.float16).reshape(8, P, 512).transpose(1, 0, 2).reshape(P, 4096)
        )

    def pack_x16(xt):  # xt: [1024, 128] f32 = X[b].T[:, :128]
        return np.ascontiguousarray(
            xt.astype(np.float16).reshape(8, P, P).transpose(1, 0, 2).reshape(P, 1024)
        )

    in_maps = []
    for b in range(B):
        x8 = pack_x(X[b].T.astype(E4M3))
        x16 = pack_x16(X[b].T[:, 0:P])
        for g in range(2):
            sl = slice(g * DH, (g + 1) * DH)
            in_maps.append(
                {
                    "xt8": x8,
                    "wq8": pack_w((Wq[:, sl] * WSCALE).astype(E4M3)),
                    "wk8": pack_w((Wk[:, sl] * WSCALE).astype(E4M3)),
                    "wv8": pack_w((Wv[:, sl] * WSCALE).astype(E4M3)),
                    "wo": pack_wo(Wo[sl, :].astype(np.float16)),
                    "wq16": pack_w16(Wq[:, sl]),
                    "wk16": pack_w16(Wk[:, sl]),
                    "wv16": pack_w16(Wv[:, sl]),
                    "xt16": x16,
                    "mask01": mask01,
                }
            )
    return in_maps


def gather_output(results, B=4):
    N = results[0]["outt"].shape[1]
    out = np.empty((B, N, D), np.float32)
    for b in range(B):
        s = (
            results[2 * b]["outt"].astype(np.float32)
            + results[2 * b + 1]["outt"].astype(np.float32)
        )
        out[b] = s.T
    return out


# ---------------------------------------------------------------------------
# Self-contained harness entry: full inputs in, full output out.
# Shards across 8 NeuronCores: core = batch b (4) x head-group g (2 x 8 heads).
# Each core runs a fused flash-style causal MHA for its 8 heads; the host
# sums the two head-group partial outputs per batch (row-parallel W_O).
# ---------------------------------------------------------------------------
from concourse.bass_utils import run_bass_kernel_spmd

_NC_CACHE = {}


def _get_nc():
    if "nc" not in _NC_CACHE:
        _NC_CACHE["nc"] = build(N=2048, interleave=True)
    return _NC_CACHE["nc"]


def kernel(X, mask, Wq, Wk, Wv, Wo):
    X = np.asarray(X, dtype=np.float32)
    mask = np.asarray(mask, dtype=np.float32)
    Wq = np.asarray(Wq, dtype=np.float32)
    Wk = np.asarray(Wk, dtype=np.float32)
    Wv = np.asarray(Wv, dtype=np.float32)
    Wo = np.asarray(Wo, dtype=np.float32)
    in_maps = make_core_inputs(X, mask, Wq, Wk, Wv, Wo)
    nc = _get_nc()
    res = run_bass_kernel_spmd(nc, in_maps, list(range(8)))
    return gather_output(res.results, B=X.shape[0])


# revision 48
# speedup vs baseline: 1.0472x; 1.0472x over previous
"""Fused causal MHA kernel for TRN2, one core = (batch b, head-group g of 8 heads).

v2: fp8-e4m3 DoubleRow projections + big DMAs + chunk-major schedule.

Layouts (per core):
  xt8   [128, (cc=4, kb=4, i=2, n=512)] fp8   X[b]^T in DoubleRow-paired
        k-blocks: element (p, cc, kb, i, n) = X[cc*512+n, kb*256+i*128+p].
        DMA'd per-cc (4 contiguous 512KB transfers).
  wq8/wk8/wv8 [128, (kb=4, i=2, m=512)] fp8   32*W column shard, paired.
  wo    [128, (dv=4, do=1024)] f16            row shard, dv on partitions.
  mask01 [128, 128] f16  triangle: mask01[j, i] = 1 if i >= j else 0.
  outt  [1024, N] f16    partial (X attn Wo_g)^T ; host sums the two
        head-group partials per batch (f32) and transposes.

Projections run in fp8 DoubleRow mode (contraction 256/instr, 2x rate):
  psQ = lhsT.T@rhs summed over the paired dim; W pre-scaled by 32 on host
  to clear e4m3 subnormals, un-scaled in the psum->sbuf copy.
Attention (S, PV) and the out-projection stay fp16.

On-chip attention per head-pair hp (2 heads):
  qt/kt [128, N] f16; partitions = (h0 d0-63, h1 d0-63); qt carries /8/32.
  v per seq m-block: [128, 8*65]; seq on partitions, (64 dims + ones) per head.
  S^T per (hp, c, jb): psum [128, 1024] = h0|h1; j on partitions, i on free.
  exp runs directly on psS (no mask add); the causal triangle is zeroed
  after exp with a fp16 multiply by mask01 (diagonal blocks only).
  PV col-packed: psO[0:65] per head (65th row = denominator via ones col).

Schedule is chunk-major: for c in 0..3: for hp in 0..3, so that Q/K
projection of chunk c+1 and the out-projection of chunk c-1 are always
available as pump work to fill the PE while the ACT engine (exp) catches up.
"""

import numpy as np
import concourse.bass as bass
import concourse.tile as tile
from concourse import bacc, mybir

F32 = mybir.dt.float32
F16 = mybir.dt.float16
F8 = mybir.dt.float8e4
AF = mybir.ActivationFunctionType
DR = mybir.MatmulPerfMode.DoubleRow

P = 128
D = 1024
DH = 512  # head-group width: 8 heads * 64
DK = 64
KB2 = 4  # 256-wide paired k-blocks
NHP = 4  # head-pairs per core
WSCALE = 32.0  # host pre-scale on W before fp8 cast


def build(N=2048, interleave=True, debug_inline_qk=False, debug_inline_v=False,
          rescue=True, rescue_level=3):
    MB = N // P  # seq 128-blocks
    MC = N // 512  # seq 512-chunks
    nc = bacc.Bacc("TRN2", target_bir_lowering=False, debug=False)

    xt8_d = nc.dram_tensor("xt8", [MC * P, 4096], F8, kind="ExternalInput")
    wq8_d = nc.dram_tensor("wq8", [P, 4096], F8, kind="ExternalInput")
    wk8_d = nc.dram_tensor("wk8", [P, 4096], F8, kind="ExternalInput")
    wv8_d = nc.dram_tensor("wv8", [P, 4096], F8, kind="ExternalInput")
    wo_d = nc.dram_tensor("wo", [P, 4096], F16, kind="ExternalInput")
    mask_d = nc.dram_tensor("mask01", [P, P], F16, kind="ExternalInput")
    # fp16 rescue inputs: W fp16 (full) + X^T fp16 for seq 0-127; the first
    # 128 output rows are recomputed in fp16 (peaked softmax amplifies the
    # fp8 projection noise there)
    wq16_d = nc.dram_tensor("wq16", [P, 4096], F16, kind="ExternalInput")
    wk16_d = nc.dram_tensor("wk16", [P, 4096], F16, kind="ExternalInput")
    wv16_d = nc.dram_tensor("wv16", [P, 4096], F16, kind="ExternalInput")
    xt16_d = nc.dram_tensor("xt16", [P, 1024], F16, kind="ExternalInput")
    out_d = nc.dram_tensor("outt", [D, N], F16, kind="ExternalOutput")

    with tile.TileContext(nc) as tc:
        with (
            tc.tile_pool(name="sb", bufs=1) as sb,
            tc.tile_pool(name="ps", bufs=1, space="PSUM") as ps,
        ):
            # ---- persistent tiles ----
            ones = sb.tile([P, DK], F16, tag="ones", bufs=1)
            mask01 = sb.tile([P, P], F16, tag="mask01", bufs=1)
            xt8 = sb.tile([P, MC * 4096], F8, tag="xt8", bufs=1)
            wv8 = sb.tile([P, 4096], F8, tag="wv8", bufs=1)
            wq8 = sb.tile([P, 4096], F8, tag="wq8", bufs=1)
            wk8 = sb.tile([P, 4096], F8, tag="wk8", bufs=1)
            wo_t = sb.tile([P, 4096], F16, tag="wo", bufs=1)
            v = [sb.tile([P, 8 * 65], F16, tag="v", bufs=MB, name=f"v{m}") for m in range(MB)]
            qt = [sb.tile([P, N], F16, tag="qt", bufs=NHP, name=f"qt{h}") for h in range(NHP)]
            kt = [sb.tile([P, N], F16, tag="kt", bufs=NHP, name=f"kt{h}") for h in range(NHP)]
            ot = [sb.tile([P, N], F16, tag="ot", bufs=NHP, name=f"ot{t}") for t in range(NHP)]

            # 5-d views: (p, cc, kb, i, n) / (p, kb, i, m)
            x5 = xt8[:].rearrange("p (cc kb i n) -> p cc kb i n", cc=MC, kb=KB2, i=2)
            wq5 = wq8[:].rearrange("p (kb i m) -> p kb i m", kb=KB2, i=2)
            wk5 = wk8[:].rearrange("p (kb i m) -> p kb i m", kb=KB2, i=2)
            wv5 = wv8[:].rearrange("p (kb i m) -> p kb i m", kb=KB2, i=2)
            wo3 = wo_t[:].rearrange("p (dv do) -> p dv do", dv=NHP)

            # ---- input DMAs: few, large, cc-ordered ----
            nc.gpsimd.memset(ones[:], 1.0)
            nc.sync.dma_start(mask01[:], mask_d.ap())
            nc.sync.dma_start(wv8[:], wv8_d.ap())
            for cc in range(MC):
                nc.sync.dma_start(
                    xt8[:, cc * 4096:(cc + 1) * 4096],
                    xt8_d.ap()[cc * P:(cc + 1) * P, :],
                )
            nc.sync.dma_start(wq8[:], wq8_d.ap())
            nc.sync.dma_start(wk8[:], wk8_d.ap())
            nc.sync.dma_start(wo_t[:], wo_d.ap())
            w16q = sb.tile([P, 4096], F16, tag="w16q", bufs=1)
            w16k = sb.tile([P, 4096], F16, tag="w16k", bufs=1)
            w16v = sb.tile([P, 4096], F16, tag="w16v", bufs=1)
            xt16 = sb.tile([P, 1024], F16, tag="xt16", bufs=1)
            nc.sync.dma_start(xt16[:], xt16_d.ap())
            nc.sync.dma_start(w16q[:], wq16_d.ap())
            nc.sync.dma_start(w16k[:], wk16_d.ap())
            nc.sync.dma_start(w16v[:], wv16_d.ap())
            wq316 = w16q[:].rearrange("p (kb m) -> p kb m", kb=8)
            wk316 = w16k[:].rearrange("p (kb m) -> p kb m", kb=8)
            wv316 = w16v[:].rearrange("p (kb m) -> p kb m", kb=8)
            x316 = xt16[:].rearrange("p (kb n) -> p kb n", kb=8)

            # warm the ACT exp table during the DMA lead-in
            warm = sb.tile([P, DK], F16, tag="warm", bufs=1, name="warm")
            nc.scalar.activation(warm[:], ones[:], AF.Exp)
            # warm the PE p-state clock with throwaway matmuls on `ones`
            psW = ps.tile([P, 512], F32, tag="proj", bufs=2, name="psW")
            for _ in range(140):
                nc.tensor.matmul(
                    psW[0:DK, 0:DK], ones[:], ones[:], start=True, stop=True
                )

            # ---- deferred work pumped between attention units ----
            # `deferred` (must-run): drained at each phase boundary so every
            # write is emitted before its next-phase reader. `opt_q`: carries
            # across phases (out-projections); popped only when `deferred`
            # is empty, which also guarantees rescue-before-outproj(0).
            deferred = []
            opt_q = []
            dve_deferred = []
            credit = [0.0]
            hold = [0]

            def pump(rate):
                if dve_deferred:
                    dve_deferred.pop(0)()
                credit[0] += rate
                while credit[0] >= 1.0:
                    if deferred:
                        deferred.pop(0)()
                    elif len(opt_q) > hold[0]:
                        opt_q.pop(0)()
                    else:
                        credit[0] = 0.0
                        break
                    credit[0] -= 1.0

            def v_proj(m):
                cc, ms = m // 4, m % 4
                psV = ps.tile([P, 512], F32, tag="proj", bufs=2, name="psV")
                for kb in range(KB2):
                    nc.tensor.matmul(
                        psV[:],
                        x5[:, cc, kb, :, ms * P:(ms + 1) * P],
                        wv5[:, kb, :, :],
                        start=(kb == 0),
                        stop=(kb == KB2 - 1),
                        perf_mode=DR,
                    )
                v3 = v[m][:].rearrange("p (h x) -> p h x", x=65)
                nc.scalar.mul(
                    v3[:, :, 0:64], psV[:].rearrange("p (h x) -> p h x", x=64),
                    1.0 / WSCALE,
                )
                nc.scalar.copy(v3[:, :, 64:65], ones[:, 0:8, None])

            def qk_proj(hp, c, w5, dst, scale):
                # one thunk per projection: a DoubleRow accumulation group
                # must not interleave with foreign (fp16) matmuls
                psQ = ps.tile([P, 512], F32, tag="proj", bufs=2, name="psQ")
                for kb in range(KB2):
                    nc.tensor.matmul(
                        psQ[:],
                        w5[:, kb, :, hp * P:(hp + 1) * P],
                        x5[:, c, kb, :, :],
                        start=(kb == 0),
                        stop=(kb == KB2 - 1),
                        perf_mode=DR,
                    )
                nc.scalar.mul(dst[:, c * 512:(c + 1) * 512], psQ[:], scale)

            def qk_work(hp, c):
                return [
                    lambda: qk_proj(hp, c, wq5, qt[hp], 0.125 / WSCALE),
                    lambda: qk_proj(hp, c, wk5, kt[hp], 1.0 / WSCALE),
                ]

            def attn_chunk(hp, c, pump_rate=0.5, norm_q=None, prepend_norm=False):
                jb_max = min(MB, 4 * c + 4)
                psOa = [
                    ps.tile([65, 512], F32, tag="psO", bufs=2, name="psO0"),
                    ps.tile([65, 512], F32, tag="psO", bufs=2, name="psO1"),
                ]
                pts = {}

                def stage_s(jb):
                    psS = ps.tile([P, 1024], F32, tag="psS", bufs=2, name="psS")
                    r = jb - 4 * c
                    pre = P * r if r > 0 else 0
                    for h2 in range(2):
                        nc.tensor.matmul(
                            psS[:, h2 * 512 + pre:(h2 + 1) * 512],
                            kt[hp][h2 * DK:(h2 + 1) * DK, jb * P:(jb + 1) * P],
                            qt[hp][h2 * DK:(h2 + 1) * DK, c * 512 + pre:(c + 1) * 512],
                            start=True,
                            stop=True,
                            tile_position=(h2 * DK, 0),
                        )
                    pt = sb.tile([P, 1024], F16, tag="pt", bufs=6, name="pt")
                    if pre:
                        # exp only the valid tail per head; zero the prefix
                        psS3 = psS[:].rearrange("p (h x) -> p h x", h=2)
                        pt3 = pt[:].rearrange("p (h x) -> p h x", h=2)
                        nc.scalar.activation(
                            pt3[:, :, pre:512], psS3[:, :, pre:512], AF.Exp
                        )
                        nc.gpsimd.memset(pt3[:, :, 0:pre], 0.0)
                    else:
                        nc.scalar.activation(pt[:], psS[:], AF.Exp)
                    if r >= 0:
                        # zero the strictly-upper part of the 128-wide
                        # diagonal triangle (post-exp 0/1 multiply)
                        for h2 in range(2):
                            nc.vector.tensor_tensor(
                                pt[:, h2 * 512 + pre:h2 * 512 + pre + P],
                                pt[:, h2 * 512 + pre:h2 * 512 + pre + P],
                                mask01[:],
                                mybir.AluOpType.mult,
                            )
                    pts[jb] = pt

                def stage_pv(jb):
                    pt = pts.pop(jb)
                    first, last = (jb == 0), (jb == jb_max - 1)
                    r = jb - 4 * c
                    pre = P * r if (r > 0 and not first) else 0
                    for h2 in range(2):
                        h = 2 * hp + h2
                        nc.tensor.matmul(
                            psOa[h2][0:65, pre:512],
                            v[jb][:, h * 65:(h + 1) * 65],
                            pt[:, h2 * 512 + pre:(h2 + 1) * 512],
                            start=first,
                            stop=last,
                            skip_group_check=True,
                        )
                    pump(pump_rate)

                for jb in range(jb_max):
                    stage_s(jb)
                    if jb >= 2:
                        stage_pv(jb - 2)
                stage_pv(jb_max - 2)
                stage_pv(jb_max - 1)

                cpO = [
                    sb.tile([65, 512], F32, tag="sm512", bufs=14, name=f"cpO{h2}")
                    for h2 in range(2)
                ]
                nc.vector.tensor_copy(cpO[0][0:65, :], psOa[0][0:65, :])
                nc.vector.tensor_copy(cpO[1][0:65, :], psOa[1][0:65, :])
                rbc = [
                    sb.tile([64, 512], F32, tag="sm512", bufs=14, name=f"rbc{h2}")
                    for h2 in range(2)
                ]
                tmp1 = sb.tile([64, 512], F16, tag="sm512h", bufs=8, name="tmp1")

                nr = sb.tile([1, 1024], F32, tag="nr", bufs=4, name="nr")

                def norm_piece(stage):
                    if stage == 0:
                        # move denominator rows (lane 64) to lane 0, then
                        # broadcast (HW broadcast source must be lane 0)
                        nc.sync.dma_start(nr[0:1, 0:512], cpO[0][64:65, :])
                        nc.sync.dma_start(nr[0:1, 512:1024], cpO[1][64:65, :])
                        nc.gpsimd.partition_broadcast(
                            rbc[0][0:64, :], nr[0:1, 0:512]
                        )
                        nc.gpsimd.partition_broadcast(
                            rbc[1][0:64, :], nr[0:1, 512:1024]
                        )
                    elif stage == 1:
                        nc.vector.reciprocal_approx_fast(
                            rbc[0][0:64, :], rbc[0][0:64, :]
                        )
                        nc.vector.reciprocal_approx_fast(
                            rbc[1][0:64, :], rbc[1][0:64, :]
                        )
                    elif stage == 2:
                        nc.vector.tensor_tensor(
                            ot[hp][0:64, c * 512:(c + 1) * 512],
                            cpO[0][0:64, :],
                            rbc[0][0:64, :],
                            mybir.AluOpType.mult,
                        )
                    elif stage == 3:
                        nc.vector.tensor_tensor(
                            tmp1[0:64, :],
                            cpO[1][0:64, :],
                            rbc[1][0:64, :],
                            mybir.AluOpType.mult,
                        )
                        nc.sync.dma_start(
                            ot[hp][64:128, c * 512:(c + 1) * 512], tmp1[0:64, :]
                        )

                if interleave and not prepend_norm:
                    for st in range(4):
                        dve_deferred.append(lambda st=st: norm_piece(st))
                else:
                    for st in range(4):
                        norm_piece(st)

            # ---- fp16 rescue of output rows 0-127 (recompute with fp16
            # Q/K/V; overwrites ot[:, 0:128] before outproj(0) reads it) ----
            rescue_state = {}

            def rescue_thunks():
                v16 = sb.tile([P, 8 * 65], F16, tag="v16", bufs=1, name="v16")
                qt16 = [
                    sb.tile([P, P], F16, tag="qk16", bufs=8, name=f"qt16_{h}")
                    for h in range(NHP)
                ]
                kt16 = [
                    sb.tile([P, P], F16, tag="qk16", bufs=8, name=f"kt16_{h}")
                    for h in range(NHP)
                ]

                def v16_proj():
                    psR = ps.tile([P, 512], F32, tag="proj", bufs=2, name="psR")
                    for kb in range(8):
                        nc.tensor.matmul(
                            psR[:],
                            x316[:, kb, :],
                            wv316[:, kb, :],
                            start=(kb == 0),
                            stop=(kb == 7),
                        )
                    v3 = v16[:].rearrange("p (h x) -> p h x", x=65)
                    nc.vector.tensor_copy(
                        v3[:, :, 0:64], psR[:].rearrange("p (h x) -> p h x", x=64)
                    )
                    nc.vector.tensor_copy(v3[:, :, 64:65], ones[:, 0:8, None])

                def qk16_proj(hp):
                    for w3, dst, scale in (
                        (wq316, qt16[hp], 0.125),
                        (wk316, kt16[hp], 1.0),
                    ):
                        psR = ps.tile([P, 512], F32, tag="proj", bufs=2, name="psR")
                        for kb in range(8):
                            nc.tensor.matmul(
                                psR[:, 0:P],
                                w3[:, kb, hp * P:(hp + 1) * P],
                                x316[:, kb, :],
                                start=(kb == 0),
                                stop=(kb == 7),
                            )
                        nc.vector.tensor_scalar_mul(dst[:], psR[:, 0:P], scale)

                def attn16(hp):
                    # concurrent row-band S matmuls must land in different
                    # PSUM banks: h2 offset by 512 f32 columns
                    psA = ps.tile([P, 1024], F32, tag="psS", bufs=2, name="psA")
                    psA3 = psA[:].rearrange("p (h x) -> p h x", h=2)
                    for h2 in range(2):
                        nc.tensor.matmul(
                            psA[:, h2 * 512:h2 * 512 + P],
                            kt16[hp][h2 * DK:(h2 + 1) * DK, :],
                            qt16[hp][h2 * DK:(h2 + 1) * DK, :],
                            start=True,
                            stop=True,
                            tile_position=(h2 * DK, 0),
                        )
                    pt16 = sb.tile([P, 256], F16, tag="pt16", bufs=2, name="pt16")
                    pt3 = pt16[:].rearrange("p (h x) -> p h x", h=2)
                    nc.scalar.activation(pt3[:, :, :], psA3[:, :, 0:P], AF.Exp)
                    for h2 in range(2):
                        nc.vector.tensor_tensor(
                            pt16[:, h2 * P:(h2 + 1) * P],
                            pt16[:, h2 * P:(h2 + 1) * P],
                            mask01[:],
                            mybir.AluOpType.mult,
                        )
                    psB = ps.tile([P, 1024], F32, tag="psS", bufs=2, name="psB")
                    psB3 = psB[:].rearrange("p (h x) -> p h x", h=2)
                    for h2 in range(2):
                        h = 2 * hp + h2
                        nc.tensor.matmul(
                            psB[0:65, h2 * 512:h2 * 512 + P],
                            v16[:, h * 65:(h + 1) * 65],
                            pt16[:, h2 * P:(h2 + 1) * P],
                            start=True,
                            stop=True,
                        )
                    cp16 = sb.tile([65, 256], F32, tag="cp16", bufs=2, name="cp16")
                    cp3 = cp16[:].rearrange("p (h x) -> p h x", h=2)
                    nc.vector.tensor_copy(cp3[0:65, :, :], psB3[0:65, :, 0:P])
                    nr16 = sb.tile([1, 256], F32, tag="nr16", bufs=2, name="nr16")
                    nr16b = sb.tile([1, 256], F32, tag="nr16", bufs=2, name="nr16b")
                    nc.sync.dma_start(nr16[0:1, 0:P], cp16[64:65, 0:P])
                    nc.sync.dma_start(nr16[0:1, P:256], cp16[64:65, P:256])
                    nc.vector.reciprocal_approx_fast(nr16b[:], nr16[:])
                    rb16 = sb.tile([64, 256], F32, tag="rb16", bufs=2, name="rb16")
                    nc.gpsimd.partition_broadcast(rb16[0:64, 0:P], nr16b[0:1, 0:P])
                    nc.gpsimd.partition_broadcast(rb16[0:64, P:256], nr16b[0:1, P:256])
                    nc.vector.tensor_tensor(
                        ot[hp][0:64, 0:P],
                        cp16[0:64, 0:P],
                        rb16[0:64, 0:P],
                        mybir.AluOpType.mult,
                    )
                    t16 = sb.tile([64, P], F16, tag="t16", bufs=2, name="t16")
                    nc.vector.tensor_tensor(
                        t16[0:64, :],
                        cp16[0:64, P:256],
                        rb16[0:64, P:256],
                        mybir.AluOpType.mult,
                    )
                    nc.sync.dma_start(ot[hp][64:128, 0:P], t16[0:64, :])

                out = [v16_proj]
                if rescue_level >= 2:
                    for hp in range(NHP):
                        out.append(lambda hp=hp: qk16_proj(hp))
                if rescue_level >= 3:
                    for hp in range(NHP):
                        out.append(lambda hp=hp: attn16(hp))
                return out

            def outproj_parts(do, c):
                cell = {}

                def part(v0, v1, fin):
                    if v0 == 0:
                        cell["ps"] = ps.tile(
                            [P, 512], F32, tag="proj", bufs=2, name="psF"
                        )
                    psF = cell["ps"]
                    for dv in range(v0, v1):
                        nc.tensor.matmul(
                            psF[:],
                            wo3[:, dv, do * P:(do + 1) * P],
                            ot[dv][:, c * 512:(c + 1) * 512],
                            start=(dv == 0),
                            stop=(dv == NHP - 1),
                        )
                    if fin:
                        o_sb = sb.tile([P, 512], F16, tag="osb", bufs=6, name="o_sb")
                        nc.scalar.copy(o_sb[:], psF[:])
                        nc.sync.dma_start(
                            out_d.ap()[do * P:(do + 1) * P, c * 512:(c + 1) * 512],
                            o_sb[:],
                        )

                return [lambda: part(0, 2, False), lambda: part(2, NHP, True)]

            # ---- schedule (chunk-major) ----
            # eager V for the first chunks (covers the xt8 DMA stream);
            # the rest is pumped during phase c=0/1.
            n_eager_v = MB if debug_inline_v else 6
            for m in range(n_eager_v):
                v_proj(m)
            deferred.extend([lambda m=m: v_proj(m) for m in range(n_eager_v, 8)])
            # Q/K for chunk 0, all head-pairs, inline
            for hp in range(NHP):
                for wfn in qk_work(hp, 0):
                    wfn()

            for c in range(MC):
                # enqueue next chunk's Q/K (+V, rescue) as must-run work;
                # previous chunk's out-projection as carryable work
                if c + 1 < MC:
                    for h in range(NHP):
                        if debug_inline_qk:
                            for wfn in qk_work(h, c + 1):
                                wfn()
                        else:
                            deferred.extend(qk_work(h, c + 1))
                if c in (1, 2) and not debug_inline_v:
                    # V blocks needed two phases ahead
                    deferred.extend(
                        [lambda m=m: v_proj(m)
                         for m in range(4 * c + 4, 4 * c + 8)]
                    )
                if c == 1 and rescue:
                    # fp16 rescue of rows 0-127: must emit before outproj(0)
                    deferred.extend(rescue_thunks())
                if c >= 1:
                    for do in range(D // P):
                        opt_q.extend(outproj_parts(do, c - 1))
                # emission burst: next chunk's Q/K copies land on the ACT
                # queue early so phase c+1's first S-pairs don't wait
                for _ in range(6):
                    if deferred:
                        deferred.pop(0)()
                phase_units = NHP * (4 * c + 4)
                done = 0
                for hp in range(NHP):
                    last = c == MC - 1 and hp == NHP - 1
                    hold[0] = 16 if c == MC - 1 else 0
                    rate = min(
                        3.0,
                        (len(deferred) + 0.5 * len(opt_q))
                        / max(phase_units - done - 4, 1)
                        + 0.5,
                    )
                    attn_chunk(hp, c, pump_rate=rate,
                               norm_q=None,
                               prepend_norm=last)
                    done += 4 * c + 4
                    if not interleave:
                        while deferred:
                            deferred.pop(0)()
                        while opt_q:
                            opt_q.pop(0)()
                # phase-boundary drain of must-run work only
                if c < MC - 1:
                    while dve_deferred:
                        dve_deferred.pop(0)()
                    while deferred:
                        deferred.pop(0)()

            # ---- drain remaining deferred work, then the last out-proj ----
            while dve_deferred:
                dve_deferred.pop(0)()
            while deferred:
                deferred.pop(0)()
            while opt_q:
                opt_q.pop(0)()
            for do in range(D // P):
                for th in outproj_parts(do, MC - 1):
                    th()

    nc.compile()
    return nc


def make_core_inputs(X, mask, Wq, Wk, Wv, Wo):
    """Full inputs -> list of 8 per-core input maps (batch-major, head-group minor)."""
    import ml_dtypes

    E4M3 = ml_dtypes.float8_e4m3fn
    B = X.shape[0]
    N = X.shape[1]
    MC = N // 512
    mask01 = np.ascontiguousarray(np.triu(np.ones((P, P), np.float16)))

    def pack_x(xt8):  # xt8: [1024, N] fp8 = X[b].T
        # (kb, i, p, cc, n) -> (cc, p, kb, i, n) -> [(cc p), (kb i n)]
        return np.ascontiguousarray(
            xt8.reshape(KB2, 2, P, MC, 512)
            .transpose(3, 2, 0, 1, 4)
            .reshape(MC * P, 4096)
        )

    def pack_w(w8):  # w8: [1024, 512] fp8 = 32*W column shard
        return np.ascontiguousarray(
            w8.reshape(KB2, 2, P, 512).transpose(2, 0, 1, 3).reshape(P, 4096)
        )

    def pack_wo(wos):  # wos: [512, 1024] f16
        return np.ascontiguousarray(
            wos.reshape(NHP, P, D).transpose(1, 0, 2).reshape(P, 4096)
        )

    def pack_w16(w):  # w: [1024, 512] f32 -> [128, (kb=8, m=512)] f16
        return np.ascontiguousarray(
            w.astype(np.float16).reshape(8, P, 512).transpose(1, 0, 2).reshape(P, 4096)
        )

    def pack_x16(xt):  # xt: [1024, 128] f32 = X[b].T[:, :128]
        return np.ascontiguousarray(
            xt.astype(np.float16).reshape(8, P, P).transpose(1, 0, 2).reshape(P, 1024)
        )

    in_maps = []
    for b in range(B):
        x8 = pack_x(X[b].T.astype(E4M3))
        x16 = pack_x16(X[b].T[:, 0:P])
        for g in range(2):
            sl = slice(g * DH, (g + 1) * DH)
            in_maps.append(
                {
                    "xt8": x8,
                    "wq8": pack_w((Wq[:, sl] * WSCALE).astype(E4M3)),
                    "wk8": pack_w((Wk[:, sl] * WSCALE).astype(E4M3)),
                    "wv8": pack_w((Wv[:, sl] * WSCALE).astype(E4M3)),
                    "wo": pack_wo(Wo[sl, :].astype(np.float16)),
                    "wq16": pack_w16(Wq[:, sl]),
                    "wk16": pack_w16(Wk[:, sl]),
                    "wv16": pack_w16(Wv[:, sl]),
                    "xt16": x16,
                    "mask01": mask01,
                }
            )
    return in_maps


def gather_output(results, B=4):
    N = results[0]["outt"].shape[1]
    out = np.empty((B, N, D), np.float32)
    for b in range(B):
        s = (
            results[2 * b]["outt"].astype(np.float32)
            + results[2 * b + 1]["outt"].astype(np.float32)
        )
        out[b] = s.T
    return out


# ---------------------------------------------------------------------------
# Self-contained harness entry: full inputs in, full output out.
# Shards across 8 NeuronCores: core = batch b (4) x head-group g (2 x 8 heads).
# Each core runs a fused flash-style causal MHA for its 8 heads; the host
# sums the two head-group partial outputs per batch (row-parallel W_O).
# ---------------------------------------------------------------------------
from concourse.bass_utils import run_bass_kernel_spmd

_NC_CACHE = {}


def _get_nc():
    if "nc" not in _NC_CACHE:
        _NC_CACHE["nc"] = build(N=2048, interleave=True)
    return _NC_CACHE["nc"]


def kernel(X, mask, Wq, Wk, Wv, Wo):
    X = np.asarray(X, dtype=np.float32)
    mask = np.asarray(mask, dtype=np.float32)
    Wq = np.asarray(Wq, dtype=np.float32)
    Wk = np.asarray(Wk, dtype=np.float32)
    Wv = np.asarray(Wv, dtype=np.float32)
    Wo = np.asarray(Wo, dtype=np.float32)
    in_maps = make_core_inputs(X, mask, Wq, Wk, Wv, Wo)
    nc = _get_nc()
    res = run_bass_kernel_spmd(nc, in_maps, list(range(8)))
    return gather_output(res.results, B=X.shape[0])


# revision 49
# speedup vs baseline: 1.2541x; 1.1976x over previous
"""Fused causal MHA kernel for TRN2, one core = (batch b, head-group g of 8 heads).

v2: fp8-e4m3 DoubleRow projections + big DMAs + chunk-major schedule.

Layouts (per core):
  xt8   [128, (cc=4, kb=4, i=2, n=512)] fp8   X[b]^T in DoubleRow-paired
        k-blocks: element (p, cc, kb, i, n) = X[cc*512+n, kb*256+i*128+p].
        DMA'd per-cc (4 contiguous 512KB transfers).
  wq8/wk8/wv8 [128, (kb=4, i=2, m=512)] fp8   32*W column shard, paired.
  wo    [128, (dv=4, do=1024)] f16            row shard, dv on partitions.
  mask01 [128, 128] f16  triangle: mask01[j, i] = 1 if i >= j else 0.
  outt  [1024, N] f16    partial (X attn Wo_g)^T ; host sums the two
        head-group partials per batch (f32) and transposes.

Projections run in fp8 DoubleRow mode (contraction 256/instr, 2x rate):
  psQ = lhsT.T@rhs summed over the paired dim; W pre-scaled by 32 on host
  to clear e4m3 subnormals, un-scaled in the psum->sbuf copy.
Attention (S, PV) and the out-projection stay fp16.

On-chip attention per head-pair hp (2 heads):
  qt/kt [128, N] f16; partitions = (h0 d0-63, h1 d0-63); qt carries /8/32.
  v per seq m-block: [128, 8*65]; seq on partitions, (64 dims + ones) per head.
  S^T per (hp, c, jb): psum [128, 1024] = h0|h1; j on partitions, i on free.
  exp runs directly on psS (no mask add); the causal triangle is zeroed
  after exp with a fp16 multiply by mask01 (diagonal blocks only).
  PV col-packed: psO[0:65] per head (65th row = denominator via ones col).

Schedule is chunk-major: for c in 0..3: for hp in 0..3, so that Q/K
projection of chunk c+1 and the out-projection of chunk c-1 are always
available as pump work to fill the PE while the ACT engine (exp) catches up.
"""

import numpy as np
import concourse.bass as bass
import concourse.tile as tile
from concourse import bacc, mybir

F32 = mybir.dt.float32
F16 = mybir.dt.float16
F8 = mybir.dt.float8e4
AF = mybir.ActivationFunctionType
DR = mybir.MatmulPerfMode.DoubleRow

P = 128
D = 1024
DH = 512  # head-group width: 8 heads * 64
DK = 64
KB2 = 4  # 256-wide paired k-blocks
NHP = 4  # head-pairs per core
WSCALE = 32.0  # host pre-scale on W before fp8 cast


def build(N=2048, interleave=True, debug_inline_qk=False, debug_inline_v=False,
          rescue=True, rescue_level=3):
    MB = N // P  # seq 128-blocks
    MC = N // 512  # seq 512-chunks
    nc = bacc.Bacc("TRN2", target_bir_lowering=False, debug=False)

    xt8_d = nc.dram_tensor("xt8", [MC * P, 4096], F8, kind="ExternalInput")
    wq8_d = nc.dram_tensor("wq8", [P, 4096], F8, kind="ExternalInput")
    wk8_d = nc.dram_tensor("wk8", [P, 4096], F8, kind="ExternalInput")
    wv8_d = nc.dram_tensor("wv8", [P, 4096], F8, kind="ExternalInput")
    wo_d = nc.dram_tensor("wo", [P, 4096], F16, kind="ExternalInput")
    mask_d = nc.dram_tensor("mask01", [P, P], F16, kind="ExternalInput")
    # fp16 rescue inputs: W fp16 (full) + X^T fp16 for seq 0-127; the first
    # 128 output rows are recomputed in fp16 (peaked softmax amplifies the
    # fp8 projection noise there)
    wq16_d = nc.dram_tensor("wq16", [P, 4096], F16, kind="ExternalInput")
    wk16_d = nc.dram_tensor("wk16", [P, 4096], F16, kind="ExternalInput")
    wv16_d = nc.dram_tensor("wv16", [P, 4096], F16, kind="ExternalInput")
    xt16_d = nc.dram_tensor("xt16", [P, 1024], F16, kind="ExternalInput")
    out_d = nc.dram_tensor("outt", [D, N], F16, kind="ExternalOutput")

    with tile.TileContext(nc) as tc:
        with (
            tc.tile_pool(name="sb", bufs=1) as sb,
            tc.tile_pool(name="ps", bufs=1, space="PSUM") as ps,
        ):
            # ---- persistent tiles ----
            ones = sb.tile([P, DK], F16, tag="ones", bufs=1)
            mask01 = sb.tile([P, P], F16, tag="mask01", bufs=1)
            xt8 = sb.tile([P, MC * 4096], F8, tag="xt8", bufs=1)
            wv8 = sb.tile([P, 4096], F8, tag="wv8", bufs=1)
            wq8 = sb.tile([P, 4096], F8, tag="wq8", bufs=1)
            wk8 = sb.tile([P, 4096], F8, tag="wk8", bufs=1)
            wo_t = sb.tile([P, 4096], F16, tag="wo", bufs=1)
            v = [sb.tile([P, 8 * 65], F16, tag="v", bufs=MB, name=f"v{m}") for m in range(MB)]
            qt = [sb.tile([P, N], F16, tag="qt", bufs=NHP, name=f"qt{h}") for h in range(NHP)]
            kt = [sb.tile([P, N], F16, tag="kt", bufs=NHP, name=f"kt{h}") for h in range(NHP)]
            ot = [sb.tile([P, N], F16, tag="ot", bufs=NHP, name=f"ot{t}") for t in range(NHP)]

            # 5-d views: (p, cc, kb, i, n) / (p, kb, i, m)
            x5 = xt8[:].rearrange("p (cc kb i n) -> p cc kb i n", cc=MC, kb=KB2, i=2)
            wq5 = wq8[:].rearrange("p (kb i m) -> p kb i m", kb=KB2, i=2)
            wk5 = wk8[:].rearrange("p (kb i m) -> p kb i m", kb=KB2, i=2)
            wv5 = wv8[:].rearrange("p (kb i m) -> p kb i m", kb=KB2, i=2)
            wo3 = wo_t[:].rearrange("p (dv do) -> p dv do", dv=NHP)

            # ---- input DMAs: few, large, cc-ordered ----
            nc.gpsimd.memset(ones[:], 1.0)
            nc.sync.dma_start(mask01[:], mask_d.ap())
            nc.sync.dma_start(wv8[:], wv8_d.ap())
            for cc in range(MC):
                nc.sync.dma_start(
                    xt8[:, cc * 4096:(cc + 1) * 4096],
                    xt8_d.ap()[cc * P:(cc + 1) * P, :],
                )
            nc.sync.dma_start(wq8[:], wq8_d.ap())
            nc.sync.dma_start(wk8[:], wk8_d.ap())
            nc.sync.dma_start(wo_t[:], wo_d.ap())
            w16q = sb.tile([P, 4096], F16, tag="w16q", bufs=1)
            w16k = sb.tile([P, 4096], F16, tag="w16k", bufs=1)
            w16v = sb.tile([P, 4096], F16, tag="w16v", bufs=1)
            xt16 = sb.tile([P, 1024], F16, tag="xt16", bufs=1)
            nc.sync.dma_start(xt16[:], xt16_d.ap())
            nc.sync.dma_start(w16q[:], wq16_d.ap())
            nc.sync.dma_start(w16k[:], wk16_d.ap())
            nc.sync.dma_start(w16v[:], wv16_d.ap())
            wq316 = w16q[:].rearrange("p (kb m) -> p kb m", kb=8)
            wk316 = w16k[:].rearrange("p (kb m) -> p kb m", kb=8)
            wv316 = w16v[:].rearrange("p (kb m) -> p kb m", kb=8)
            x316 = xt16[:].rearrange("p (kb n) -> p kb n", kb=8)

            # warm the ACT exp table during the DMA lead-in
            warm = sb.tile([P, DK], F16, tag="warm", bufs=1, name="warm")
            nc.scalar.activation(warm[:], ones[:], AF.Exp)
            # warm the PE p-state clock with throwaway matmuls on `ones`
            psW = ps.tile([P, 512], F32, tag="proj", bufs=2, name="psW")
            for _ in range(140):
                nc.tensor.matmul(
                    psW[0:DK, 0:DK], ones[:], ones[:], start=True, stop=True
                )

            # ---- deferred work pumped between attention units ----
            # `deferred` (must-run): drained at each phase boundary so every
            # write is emitted before its next-phase reader. `opt_q`: carries
            # across phases (out-projections); popped only when `deferred`
            # is empty, which also guarantees rescue-before-outproj(0).
            deferred = []
            opt_q = []
            dve_deferred = []
            credit = [0.0]
            hold = [0]

            def pump(rate):
                for _ in range(2):
                    if dve_deferred:
                        dve_deferred.pop(0)()
                credit[0] += rate
                while credit[0] >= 1.0:
                    if deferred:
                        deferred.pop(0)()
                    elif len(opt_q) > hold[0]:
                        opt_q.pop(0)()
                    else:
                        credit[0] = 0.0
                        break
                    credit[0] -= 1.0

            def v_proj(m):
                cc, ms = m // 4, m % 4
                psV = ps.tile([P, 512], F32, tag="proj", bufs=2, name="psV")
                for kb in range(KB2):
                    nc.tensor.matmul(
                        psV[:],
                        x5[:, cc, kb, :, ms * P:(ms + 1) * P],
                        wv5[:, kb, :, :],
                        start=(kb == 0),
                        stop=(kb == KB2 - 1),
                        perf_mode=DR,
                    )
                v3 = v[m][:].rearrange("p (h x) -> p h x", x=65)
                nc.scalar.mul(
                    v3[:, :, 0:64], psV[:].rearrange("p (h x) -> p h x", x=64),
                    1.0 / WSCALE,
                )
                nc.scalar.copy(v3[:, :, 64:65], ones[:, 0:8, None])

            def qk_proj(hp, c, w5, dst, scale):
                # one thunk per projection: a DoubleRow accumulation group
                # must not interleave with foreign (fp16) matmuls
                psQ = ps.tile([P, 512], F32, tag="proj", bufs=2, name="psQ")
                for kb in range(KB2):
                    nc.tensor.matmul(
                        psQ[:],
                        w5[:, kb, :, hp * P:(hp + 1) * P],
                        x5[:, c, kb, :, :],
                        start=(kb == 0),
                        stop=(kb == KB2 - 1),
                        perf_mode=DR,
                    )
                nc.scalar.mul(dst[:, c * 512:(c + 1) * 512], psQ[:], scale)

            def qk_work(hp, c):
                return [
                    lambda: qk_proj(hp, c, wq5, qt[hp], 0.125 / WSCALE),
                    lambda: qk_proj(hp, c, wk5, kt[hp], 1.0 / WSCALE),
                ]

            def attn_chunk(hp, c, pump_rate=0.5, norm_q=None, prepend_norm=False):
                jb_max = min(MB, 4 * c + 4)
                psOa = [
                    ps.tile([65, 512], F32, tag="psO", bufs=2, name="psO0"),
                    ps.tile([65, 512], F32, tag="psO", bufs=2, name="psO1"),
                ]
                pts = {}

                def stage_s(jb):
                    psS = ps.tile([P, 1024], F32, tag="psS", bufs=2, name="psS")
                    r = jb - 4 * c
                    pre = P * r if r > 0 else 0
                    for h2 in range(2):
                        nc.tensor.matmul(
                            psS[:, h2 * 512 + pre:(h2 + 1) * 512],
                            kt[hp][h2 * DK:(h2 + 1) * DK, jb * P:(jb + 1) * P],
                            qt[hp][h2 * DK:(h2 + 1) * DK, c * 512 + pre:(c + 1) * 512],
                            start=True,
                            stop=True,
                            tile_position=(h2 * DK, 0),
                        )
                    pt = sb.tile([P, 1024], F16, tag="pt", bufs=6, name="pt")
                    if pre:
                        # exp only the valid tail per head; zero the prefix
                        psS3 = psS[:].rearrange("p (h x) -> p h x", h=2)
                        pt3 = pt[:].rearrange("p (h x) -> p h x", h=2)
                        nc.scalar.activation(
                            pt3[:, :, pre:512], psS3[:, :, pre:512], AF.Exp
                        )
                        nc.gpsimd.memset(pt3[:, :, 0:pre], 0.0)
                    else:
                        nc.scalar.activation(pt[:], psS[:], AF.Exp)
                    if r >= 0:
                        # zero the strictly-upper part of the 128-wide
                        # diagonal triangle (post-exp 0/1 multiply)
                        for h2 in range(2):
                            nc.vector.tensor_tensor(
                                pt[:, h2 * 512 + pre:h2 * 512 + pre + P],
                                pt[:, h2 * 512 + pre:h2 * 512 + pre + P],
                                mask01[:],
                                mybir.AluOpType.mult,
                            )
                    pts[jb] = pt

                def stage_pv(jb):
                    pt = pts.pop(jb)
                    first, last = (jb == 0), (jb == jb_max - 1)
                    r = jb - 4 * c
                    pre = P * r if (r > 0 and not first) else 0
                    for h2 in range(2):
                        h = 2 * hp + h2
                        nc.tensor.matmul(
                            psOa[h2][0:65, pre:512],
                            v[jb][:, h * 65:(h + 1) * 65],
                            pt[:, h2 * 512 + pre:(h2 + 1) * 512],
                            start=first,
                            stop=last,
                            skip_group_check=True,
                        )
                    pump(pump_rate)

                for jb in range(jb_max):
                    stage_s(jb)
                    if jb >= 2:
                        stage_pv(jb - 2)
                stage_pv(jb_max - 2)
                stage_pv(jb_max - 1)

                cpO = [
                    sb.tile([65, 512], F32, tag="sm512", bufs=14, name=f"cpO{h2}")
                    for h2 in range(2)
                ]
                nc.vector.tensor_copy(cpO[0][0:65, :], psOa[0][0:65, :])
                nc.vector.tensor_copy(cpO[1][0:65, :], psOa[1][0:65, :])
                rbc = [
                    sb.tile([64, 512], F32, tag="sm512", bufs=14, name=f"rbc{h2}")
                    for h2 in range(2)
                ]
                tmp1 = sb.tile([64, 512], F16, tag="sm512h", bufs=8, name="tmp1")

                nr = sb.tile([1, 1024], F32, tag="nr", bufs=4, name="nr")

                def norm_piece(stage):
                    if stage == 0:
                        # move denominator rows (lane 64) to lane 0, then
                        # broadcast (HW broadcast source must be lane 0)
                        nc.sync.dma_start(nr[0:1, 0:512], cpO[0][64:65, :])
                        nc.sync.dma_start(nr[0:1, 512:1024], cpO[1][64:65, :])
                        nc.gpsimd.partition_broadcast(
                            rbc[0][0:64, :], nr[0:1, 0:512]
                        )
                        nc.gpsimd.partition_broadcast(
                            rbc[1][0:64, :], nr[0:1, 512:1024]
                        )
                    elif stage == 1:
                        nc.vector.reciprocal_approx_fast(
                            rbc[0][0:64, :], rbc[0][0:64, :]
                        )
                        nc.vector.reciprocal_approx_fast(
                            rbc[1][0:64, :], rbc[1][0:64, :]
                        )
                    elif stage == 2:
                        nc.vector.tensor_tensor(
                            ot[hp][0:64, c * 512:(c + 1) * 512],
                            cpO[0][0:64, :],
                            rbc[0][0:64, :],
                            mybir.AluOpType.mult,
                        )
                    elif stage == 3:
                        nc.vector.tensor_tensor(
                            tmp1[0:64, :],
                            cpO[1][0:64, :],
                            rbc[1][0:64, :],
                            mybir.AluOpType.mult,
                        )
                        nc.sync.dma_start(
                            ot[hp][64:128, c * 512:(c + 1) * 512], tmp1[0:64, :]
                        )

                if interleave and not prepend_norm:
                    for st in range(4):
                        dve_deferred.append(lambda st=st: norm_piece(st))
                else:
                    for st in range(4):
                        norm_piece(st)

            # ---- fp16 rescue of output rows 0-127 (recompute with fp16
            # Q/K/V; overwrites ot[:, 0:128] before outproj(0) reads it) ----
            rescue_state = {}

            def rescue_thunks():
                v16 = sb.tile([P, 8 * 65], F16, tag="v16", bufs=1, name="v16")
                qt16 = [
                    sb.tile([P, P], F16, tag="qk16", bufs=8, name=f"qt16_{h}")
                    for h in range(NHP)
                ]
                kt16 = [
                    sb.tile([P, P], F16, tag="qk16", bufs=8, name=f"kt16_{h}")
                    for h in range(NHP)
                ]

                def v16_proj():
                    psR = ps.tile([P, 512], F32, tag="proj", bufs=2, name="psR")
                    for kb in range(8):
                        nc.tensor.matmul(
                            psR[:],
                            x316[:, kb, :],
                            wv316[:, kb, :],
                            start=(kb == 0),
                            stop=(kb == 7),
                        )
                    v3 = v16[:].rearrange("p (h x) -> p h x", x=65)
                    nc.vector.tensor_copy(
                        v3[:, :, 0:64], psR[:].rearrange("p (h x) -> p h x", x=64)
                    )
                    nc.vector.tensor_copy(v3[:, :, 64:65], ones[:, 0:8, None])

                def qk16_proj(hp):
                    for w3, dst, scale in (
                        (wq316, qt16[hp], 0.125),
                        (wk316, kt16[hp], 1.0),
                    ):
                        psR = ps.tile([P, 512], F32, tag="proj", bufs=2, name="psR")
                        for kb in range(8):
                            nc.tensor.matmul(
                                psR[:, 0:P],
                                w3[:, kb, hp * P:(hp + 1) * P],
                                x316[:, kb, :],
                                start=(kb == 0),
                                stop=(kb == 7),
                            )
                        nc.vector.tensor_scalar_mul(dst[:], psR[:, 0:P], scale)

                def attn16(hp):
                    # concurrent row-band S matmuls must land in different
                    # PSUM banks: h2 offset by 512 f32 columns
                    psA = ps.tile([P, 1024], F32, tag="psS", bufs=2, name="psA")
                    psA3 = psA[:].rearrange("p (h x) -> p h x", h=2)
                    for h2 in range(2):
                        nc.tensor.matmul(
                            psA[:, h2 * 512:h2 * 512 + P],
                            kt16[hp][h2 * DK:(h2 + 1) * DK, :],
                            qt16[hp][h2 * DK:(h2 + 1) * DK, :],
                            start=True,
                            stop=True,
                            tile_position=(h2 * DK, 0),
                        )
                    pt16 = sb.tile([P, 256], F16, tag="pt16", bufs=2, name="pt16")
                    pt3 = pt16[:].rearrange("p (h x) -> p h x", h=2)
                    nc.scalar.activation(pt3[:, :, :], psA3[:, :, 0:P], AF.Exp)
                    for h2 in range(2):
                        nc.vector.tensor_tensor(
                            pt16[:, h2 * P:(h2 + 1) * P],
                            pt16[:, h2 * P:(h2 + 1) * P],
                            mask01[:],
                            mybir.AluOpType.mult,
                        )
                    psB = ps.tile([P, 1024], F32, tag="psS", bufs=2, name="psB")
                    psB3 = psB[:].rearrange("p (h x) -> p h x", h=2)
                    for h2 in range(2):
                        h = 2 * hp + h2
                        nc.tensor.matmul(
                            psB[0:65, h2 * 512:h2 * 512 + P],
                            v16[:, h * 65:(h + 1) * 65],
                            pt16[:, h2 * P:(h2 + 1) * P],
                            start=True,
                            stop=True,
                        )
                    cp16 = sb.tile([65, 256], F32, tag="cp16", bufs=2, name="cp16")
                    cp3 = cp16[:].rearrange("p (h x) -> p h x", h=2)
                    nc.vector.tensor_copy(cp3[0:65, :, :], psB3[0:65, :, 0:P])
                    nr16 = sb.tile([1, 256], F32, tag="nr16", bufs=2, name="nr16")
                    nr16b = sb.tile([1, 256], F32, tag="nr16", bufs=2, name="nr16b")
                    nc.sync.dma_start(nr16[0:1, 0:P], cp16[64:65, 0:P])
                    nc.sync.dma_start(nr16[0:1, P:256], cp16[64:65, P:256])
                    nc.vector.reciprocal_approx_fast(nr16b[:], nr16[:])
                    rb16 = sb.tile([64, 256], F32, tag="rb16", bufs=2, name="rb16")
                    nc.gpsimd.partition_broadcast(rb16[0:64, 0:P], nr16b[0:1, 0:P])
                    nc.gpsimd.partition_broadcast(rb16[0:64, P:256], nr16b[0:1, P:256])
                    nc.vector.tensor_tensor(
                        ot[hp][0:64, 0:P],
                        cp16[0:64, 0:P],
                        rb16[0:64, 0:P],
                        mybir.AluOpType.mult,
                    )
                    t16 = sb.tile([64, P], F16, tag="t16", bufs=2, name="t16")
                    nc.vector.tensor_tensor(
                        t16[0:64, :],
                        cp16[0:64, P:256],
                        rb16[0:64, P:256],
                        mybir.AluOpType.mult,
                    )
                    nc.sync.dma_start(ot[hp][64:128, 0:P], t16[0:64, :])

                out = [v16_proj]
                if rescue_level >= 2:
                    for hp in range(NHP):
                        out.append(lambda hp=hp: qk16_proj(hp))
                if rescue_level >= 3:
                    for hp in range(NHP):
                        out.append(lambda hp=hp: attn16(hp))
                return out

            def outproj_parts(do, c):
                cell = {}

                def part(v0, v1, fin):
                    if v0 == 0:
                        cell["ps"] = ps.tile(
                            [P, 512], F32, tag="proj", bufs=2, name="psF"
                        )
                    psF = cell["ps"]
                    for dv in range(v0, v1):
                        nc.tensor.matmul(
                            psF[:],
                            wo3[:, dv, do * P:(do + 1) * P],
                            ot[dv][:, c * 512:(c + 1) * 512],
                            start=(dv == 0),
                            stop=(dv == NHP - 1),
                        )
                    if fin:
                        o_sb = sb.tile([P, 512], F16, tag="osb", bufs=6, name="o_sb")
                        nc.scalar.copy(o_sb[:], psF[:])
                        nc.sync.dma_start(
                            out_d.ap()[do * P:(do + 1) * P, c * 512:(c + 1) * 512],
                            o_sb[:],
                        )

                return [lambda: part(0, 2, False), lambda: part(2, NHP, True)]

            # ---- schedule (chunk-major) ----
            # eager V for the first chunks (covers the xt8 DMA stream);
            # the rest is pumped during phase c=0/1.
            n_eager_v = MB if debug_inline_v else 6
            for m in range(n_eager_v):
                v_proj(m)
            deferred.extend([lambda m=m: v_proj(m) for m in range(n_eager_v, 8)])
            # Q/K for chunk 0, all head-pairs, inline
            for hp in range(NHP):
                for wfn in qk_work(hp, 0):
                    wfn()

            for c in range(MC):
                # enqueue next chunk's Q/K (+V, rescue) as must-run work;
                # previous chunk's out-projection as carryable work
                if c + 1 < MC:
                    for h in range(NHP):
                        if debug_inline_qk:
                            for wfn in qk_work(h, c + 1):
                                wfn()
                        else:
                            deferred.extend(qk_work(h, c + 1))
                if c in (1, 2) and not debug_inline_v:
                    # V blocks needed two phases ahead
                    deferred.extend(
                        [lambda m=m: v_proj(m)
                         for m in range(4 * c + 4, 4 * c + 8)]
                    )
                if c == 1 and rescue:
                    # fp16 rescue of rows 0-127: must emit before outproj(0)
                    deferred.extend(rescue_thunks())
                if c >= 1:
                    for do in range(D // P):
                        opt_q.extend(outproj_parts(do, c - 1))
                # emission burst: next chunk's Q/K copies land on the ACT
                # queue early so phase c+1's first S-pairs don't wait
                for _ in range(6):
                    if deferred:
                        deferred.pop(0)()
                phase_units = NHP * (4 * c + 4)
                done = 0
                for hp in range(NHP):
                    last = c == MC - 1 and hp == NHP - 1
                    hold[0] = 16 if c == MC - 1 else 0
                    rate = min(
                        3.0,
                        (len(deferred) + 0.5 * len(opt_q))
                        / max(phase_units - done - 4, 1)
                        + 0.5,
                    )
                    attn_chunk(hp, c, pump_rate=rate,
                               norm_q=None,
                               prepend_norm=last)
                    done += 4 * c + 4
                    if not interleave:
                        while deferred:
                            deferred.pop(0)()
                        while opt_q:
                            opt_q.pop(0)()
                # phase-boundary drain of must-run work only
                if c < MC - 1:
                    while dve_deferred:
                        dve_deferred.pop(0)()
                    while deferred:
                        deferred.pop(0)()

            # ---- drain remaining deferred work, then the last out-proj ----
            while dve_deferred:
                dve_deferred.pop(0)()
            while deferred:
                deferred.pop(0)()
            while opt_q:
                opt_q.pop(0)()
            for do in range(D // P):
                for th in outproj_parts(do, MC - 1):
                    th()

    nc.compile()
    return nc


def make_core_inputs(X, mask, Wq, Wk, Wv, Wo):
    """Full inputs -> list of 8 per-core input maps (batch-major, head-group minor)."""
    import ml_dtypes

    E4M3 = ml_dtypes.float8_e4m3fn
    B = X.shape[0]
    N = X.shape[1]
    MC = N // 512
    mask01 = np.ascontiguousarray(np.triu(np.ones((P, P), np.float16)))

    def pack_x(xt8):  # xt8: [1024, N] fp8 = X[b].T
        # (kb, i, p, cc, n) -> (cc, p, kb, i, n) -> [(cc p), (kb i n)]
        return np.ascontiguousarray(
            xt8.reshape(KB2, 2, P, MC, 512)
            .transpose(3, 2, 0, 1, 4)
            .reshape(MC * P, 4096)
        )

    def pack_w(w8):  # w8: [1024, 512] fp8 = 32*W column shard
        return np.ascontiguousarray(
            w8.reshape(KB2, 2, P, 512).transpose(2, 0, 1, 3).reshape(P, 4096)
        )

    def pack_wo(wos):  # wos: [512, 1024] f16
        return np.ascontiguousarray(
            wos.reshape(NHP, P, D).transpose(1, 0, 2).reshape(P, 4096)
        )

    def pack_w16(w):  # w: [1024, 512] f32 -> [128, (kb=8, m=512)] f16
        return np.ascontiguousarray(
            w.astype(np.float16).reshape(8, P, 512).transpose(1, 0, 2).reshape(P, 4096)
        )

    def pack_x16(xt):  # xt: [1024, 128] f32 = X[b].T[:, :128]
        return np.ascontiguousarray(
            xt.astype(np.float16).reshape(8, P, P).transpose(1, 0, 2).reshape(P, 1024)
        )

    in_maps = []
    for b in range(B):
        x8 = pack_x(X[b].T.astype(E4M3))
        x16 = pack_x16(X[b].T[:, 0:P])
        for g in range(2):
            sl = slice(g * DH, (g + 1) * DH)
            in_maps.append(
                {
                    "xt8": x8,
                    "wq8": pack_w((Wq[:, sl] * WSCALE).astype(E4M3)),
                    "wk8": pack_w((Wk[:, sl] * WSCALE).astype(E4M3)),
                    "wv8": pack_w((Wv[:, sl] * WSCALE).astype(E4M3)),
                    "wo": pack_wo(Wo[sl, :].astype(np.float16)),
                    "wq16": pack_w16(Wq[:, sl]),
                    "wk16": pack_w16(Wk[:, sl]),
                    "wv16": pack_w16(Wv[:, sl]),
                    "xt16": x16,
                    "mask01": mask01,
                }
            )
    return in_maps


def gather_output(results, B=4):
    N = results[0]["outt"].shape[1]
    out = np.empty((B, N, D), np.float32)
    for b in range(B):
        s = (
            results[2 * b]["outt"].astype(np.float32)
            + results[2 * b + 1]["outt"].astype(np.float32)
        )
        out[b] = s.T
    return out


# ---------------------------------------------------------------------------
# Self-contained harness entry: full inputs in, full output out.
# Shards across 8 NeuronCores: core = batch b (4) x head-group g (2 x 8 heads).
# Each core runs a fused flash-style causal MHA for its 8 heads; the host
# sums the two head-group partial outputs per batch (row-parallel W_O).
# ---------------------------------------------------------------------------
from concourse.bass_utils import run_bass_kernel_spmd

_NC_CACHE = {}


def _get_nc():
    if "nc" not in _NC_CACHE:
        _NC_CACHE["nc"] = build(N=2048, interleave=True)
    return _NC_CACHE["nc"]


def kernel(X, mask, Wq, Wk, Wv, Wo):
    X = np.asarray(X, dtype=np.float32)
    mask = np.asarray(mask, dtype=np.float32)
    Wq = np.asarray(Wq, dtype=np.float32)
    Wk = np.asarray(Wk, dtype=np.float32)
    Wv = np.asarray(Wv, dtype=np.float32)
    Wo = np.asarray(Wo, dtype=np.float32)
    in_maps = make_core_inputs(X, mask, Wq, Wk, Wv, Wo)
    nc = _get_nc()
    res = run_bass_kernel_spmd(nc, in_maps, list(range(8)))
    return gather_output(res.results, B=X.shape[0])
